# revision 15
# baseline (speedup 1.0000x reference)
"""NT-Xent loss on 8 TRN2 NeuronCores.

Reference computes, for z = concat(z1, z2) (2N=8192 rows, D=256):
    zn  = z / max(||z||, eps)
    sim = (zn @ zn.T) / T, diag masked to -1e9
    loss = mean_i( logsumexp_j sim[i, j] - sim[i, pos(i)] ),  pos(i) = (i + N) % 2N

Sharding: 2N rows split into 8 blocks of 1024. Each core computes its
1024x8192 row-block of sim against the full replicated zn.T, fused with
exp+rowsum on ScalarE (accum_out), so the sim matrix is never
materialized in HBM.

SPMD trick: core c receives zn.T with columns rotated left by c*1024, so
on EVERY core its own rows sit at columns 0:1024 and the positive
partners at columns 4096:5120. All diagonal-window access patterns are
then compile-time constants, identical across cores; only the data
differs. The self-similarity diagonal is masked pre-exp with a -1e5*I
tile; the positive logit is extracted with an eye-masked
tensor_tensor_reduce. log() and the final mean run on host (tiny).
"""

import sys

if "/opt/trn_rl_repo" not in sys.path:
    sys.path.insert(0, "/opt/trn_rl_repo")

import ml_dtypes
import numpy as np

import concourse.bass as bass
import concourse.mybir as mybir
import concourse.tile as tile
from concourse import bacc
from concourse.bass_utils import run_bass_kernel_spmd

N = 4096
D = 256
TWO_N = 2 * N          # 8192
TEMPERATURE = 0.07
EPS = 1e-8
N_CORES = 8
ROWS_PER_CORE = TWO_N // N_CORES   # 1024
M_TILES = ROWS_PER_CORE // 128     # 8 row-tiles of 128
CB = 2048                          # psum / column-block width
N_CB = TWO_N // CB                 # 4 column blocks
POS_CB = N // CB                   # column block holding the positives (2)

_cached = {}


def _build_bass(m_tiles=M_TILES):
    f32 = mybir.dt.float32
    bf16 = mybir.dt.bfloat16
    nc = bacc.Bacc("TRN2", target_bir_lowering=False, debug=False)

    znt = nc.declare_dram_parameter("znt", [D, TWO_N], bf16, isOutput=False)
    eye = nc.declare_dram_parameter("eye", [128, 128], f32, isOutput=False)
    s_out = nc.declare_dram_parameter("S", [128, m_tiles], f32, isOutput=True)
    sexp_out = nc.declare_dram_parameter("sexp", [128, m_tiles], f32, isOutput=True)
    pexp_out = nc.declare_dram_parameter("pexp", [128, m_tiles], f32, isOutput=True)

    with tile.TileContext(nc) as tc:
        with (
            tc.tile_pool(name="zchunks", bufs=1) as zpool,
            tc.tile_pool(name="consts", bufs=1) as cpool,
            tc.tile_pool(name="stats", bufs=1) as spool,
            tc.tile_pool(name="discard", bufs=3) as dpool,
            tc.tile_pool(name="scratch", bufs=2) as scpool,
            tc.tile_pool(name="psum", bufs=2, space=bass.MemorySpace.PSUM) as ppool,
        ):
            # Column-chunked copies of znt: zt[k][cb] holds rows k*128:(k+1)*128,
            # cols cb*2048:(cb+1)*2048. Separate tiles => independent DMA deps,
            # so phase cb only waits on its own chunks.
            zt = [[None] * N_CB for _ in range(2)]
            for cb in range(N_CB):
                for k in range(2):
                    t = zpool.tile([128, CB], bf16, tag=f"z{k}_{cb}")
                    nc.sync.dma_start(t[:], znt[k * 128 : (k + 1) * 128, cb * CB : (cb + 1) * CB])
                    zt[k][cb] = t

            eye_t = cpool.tile([128, 128], f32, tag="eye")
            nc.sync.dma_start(eye_t[:], eye[:])

            acc = spool.tile([128, m_tiles * N_CB], f32, tag="acc")
            s_t = spool.tile([128, m_tiles], f32, tag="S")
            sexp_t = spool.tile([128, m_tiles], f32, tag="sexp")
            pexp_t = spool.tile([128, m_tiles], f32, tag="pexp")

            for cb in range(N_CB):
                for m in range(m_tiles):
                    moff = m * 128
                    ps = ppool.tile([128, CB], f32, tag="ps")
                    for k in range(2):
                        for nn in range(CB // 512):
                            nc.tensor.matmul(
                                ps[:, nn * 512 : (nn + 1) * 512],
                                lhsT=zt[k][0][:, moff : moff + 128],
                                rhs=zt[k][cb][:, nn * 512 : (nn + 1) * 512],
                                start=(k == 0),
                                stop=(k == 1),
                            )
                    ex = dpool.tile([128, CB], f32, tag="ex")
                    nc.scalar.activation(
                        out=ex[:],
                        in_=ps[:],
                        func=mybir.ActivationFunctionType.Exp,
                        bias=0.0,
                        scale=1.0 / TEMPERATURE,
                        accum_out=acc[:, m * N_CB + cb : m * N_CB + cb + 1],
                    )
                    # extract exp'd diagonals from the SBUF exp tile:
                    # self-sim diag lives in cb 0 at cols moff:moff+128,
                    # positive-pair diag in cb POS_CB at the same offset.
                    for cond, dst in ((cb == 0, sexp_t), (cb == POS_CB, pexp_t)):
                        if cond:
                            poff = moff  # 4096 % CB == 0: same offset in cb 0 and cb 2
                            sc = scpool.tile([128, 128], f32, tag="sc")
                            nc.vector.tensor_tensor(
                                sc[:],
                                ex[:, poff : poff + 128],
                                eye_t[:],
                                mybir.AluOpType.mult,
                            )
                            nc.vector.reduce_sum(
                                dst[:, m : m + 1], sc[:], axis=mybir.AxisListType.X
                            )

            for m in range(m_tiles):
                nc.vector.reduce_sum(
                    s_t[:, m : m + 1],
                    acc[:, m * N_CB : (m + 1) * N_CB],
                    axis=mybir.AxisListType.X,
                )

            nc.sync.dma_start(s_out[:], s_t[:])
            nc.sync.dma_start(sexp_out[:], sexp_t[:])
            nc.sync.dma_start(pexp_out[:], pexp_t[:])

    nc.compile()
    return nc


def _prepare_inputs(z1, z2):
    z = np.concatenate([np.asarray(z1), np.asarray(z2)], axis=0).astype(np.float32)
    norms = np.maximum(np.sqrt((z.astype(np.float64) ** 2).sum(-1)), EPS)
    zn = (z / norms[:, None]).astype(np.float32)
    znb = zn.astype(ml_dtypes.bfloat16)
    znt = np.ascontiguousarray(znb.T)  # [D, 2N]
    eye = np.eye(128, dtype=np.float32)
    in_maps = []
    for c in range(N_CORES):
        znt_c = np.ascontiguousarray(np.roll(znt, -c * ROWS_PER_CORE, axis=1))
        in_maps.append({"znt": znt_c, "eye": eye})
    return in_maps


def kernel(z1, z2):
    if "nc" not in _cached:
        _cached["nc"] = _build_bass()
    nc = _cached["nc"]
    in_maps = _prepare_inputs(z1, z2)
    res = run_bass_kernel_spmd(nc, in_maps, core_ids=list(range(N_CORES)))
    results = res.results

    per_row_loss = np.zeros(TWO_N, dtype=np.float64)
    for c in range(N_CORES):
        # [128, M_TILES]; element [l, m] belongs to global row c*1024 + m*128 + l
        S = np.asarray(results[c]["S"], dtype=np.float64)
        sexp = np.asarray(results[c]["sexp"], dtype=np.float64)
        pexp = np.asarray(results[c]["pexp"], dtype=np.float64)
        # drop the self-similarity term from the softmax denominator, then
        # loss_i = log(sum_{j!=i} exp(sim/T)) - pos/T
        rows = np.log((S - sexp).T.reshape(-1)) - np.log(pexp.T.reshape(-1))
        per_row_loss[c * ROWS_PER_CORE : (c + 1) * ROWS_PER_CORE] = rows
    return np.float32(per_row_loss.mean())


# revision 19
# speedup vs baseline: 1.0148x; 1.0148x over previous
"""NT-Xent loss on 8 TRN2 NeuronCores.

Reference computes, for z = concat(z1, z2) (2N=8192 rows, D=256):
    zn  = z / max(||z||, eps)
    sim = (zn @ zn.T) / T, diag masked to -1e9
    loss = mean_i( logsumexp_j sim[i, j] - sim[i, pos(i)] ),  pos(i) = (i + N) % 2N

Sharding: 2N rows split into 8 blocks of 1024. Each core computes its
1024x8192 row-block of sim against the full replicated zn.T, fused with
exp+rowsum on ScalarE (accum_out), so the sim matrix is never
materialized in HBM.

SPMD trick: core c receives zn.T with columns rotated left by c*1024, so
on EVERY core its own rows sit at columns 0:1024 and the positive
partners at columns 4096:5120. All diagonal-window access patterns are
then compile-time constants, identical across cores; only the data
differs. The exp'd self-similarity and positive-pair diagonals are
extracted from the ScalarE output tile with an eye-mask multiply +
reduce on VectorE; the host subtracts exp(self/T) from the denominator
sum and computes log() and the final mean (tiny).
"""

import sys

if "/opt/trn_rl_repo" not in sys.path:
    sys.path.insert(0, "/opt/trn_rl_repo")

import ml_dtypes
import numpy as np

import concourse.bass as bass
import concourse.mybir as mybir
import concourse.tile as tile
from concourse import bacc
from concourse.bass_utils import run_bass_kernel_spmd

N = 4096
D = 256
TWO_N = 2 * N          # 8192
TEMPERATURE = 0.07
EPS = 1e-8
N_CORES = 8
ROWS_PER_CORE = TWO_N // N_CORES   # 1024
M_TILES = ROWS_PER_CORE // 128     # 8 row-tiles of 128
CB = 2048                          # psum / column-block width
N_CB = TWO_N // CB                 # 4 column blocks
POS_CB = N // CB                   # column block holding the positives (2)

_cached = {}


def _build_bass(m_tiles=M_TILES):
    f32 = mybir.dt.float32
    bf16 = mybir.dt.bfloat16
    nc = bacc.Bacc("TRN2", target_bir_lowering=False, debug=False)

    znt = nc.declare_dram_parameter("znt", [D, TWO_N], bf16, isOutput=False)
    eye = nc.declare_dram_parameter("eye", [128, 128], f32, isOutput=False)
    s_out = nc.declare_dram_parameter("S", [128, m_tiles], f32, isOutput=True)
    sexp_out = nc.declare_dram_parameter("sexp", [128, m_tiles], f32, isOutput=True)
    pexp_out = nc.declare_dram_parameter("pexp", [128, m_tiles], f32, isOutput=True)

    with tile.TileContext(nc) as tc:
        with (
            tc.tile_pool(name="zchunks", bufs=1) as zpool,
            tc.tile_pool(name="consts", bufs=1) as cpool,
            tc.tile_pool(name="stats", bufs=1) as spool,
            tc.tile_pool(name="discard", bufs=4) as dpool,
            tc.tile_pool(name="scratch", bufs=2) as scpool,
            tc.tile_pool(name="psum", bufs=2, space=bass.MemorySpace.PSUM) as ppool,
        ):
            # Column-chunked copies of znt: zt[k][cb] holds rows k*128:(k+1)*128,
            # cols cb*2048:(cb+1)*2048. Separate tiles => independent DMA deps,
            # so phase cb only waits on its own chunks.
            zt = [[None] * N_CB for _ in range(2)]
            for cb in range(N_CB):
                for k in range(2):
                    t = zpool.tile([128, CB], bf16, tag=f"z{k}_{cb}")
                    # alternate DMA issue engines so descriptor issue isn't
                    # serialized on one queue (head-latency win)
                    eng = nc.sync if (cb * 2 + k) % 2 == 0 else nc.gpsimd
                    eng.dma_start(t[:], znt[k * 128 : (k + 1) * 128, cb * CB : (cb + 1) * CB])
                    zt[k][cb] = t

            eye_t = cpool.tile([128, 128], f32, tag="eye")
            nc.sync.dma_start(eye_t[:], eye[:])

            acc = spool.tile([128, m_tiles * N_CB], f32, tag="acc")
            s_t = spool.tile([128, m_tiles], f32, tag="S")
            sexp_t = spool.tile([128, m_tiles], f32, tag="sexp")
            pexp_t = spool.tile([128, m_tiles], f32, tag="pexp")

            for cb in range(N_CB):
                for m in range(m_tiles):
                    moff = m * 128
                    ps = ppool.tile([128, CB], f32, tag="ps")
                    for k in range(2):
                        for nn in range(CB // 512):
                            nc.tensor.matmul(
                                ps[:, nn * 512 : (nn + 1) * 512],
                                lhsT=zt[k][0][:, moff : moff + 128],
                                rhs=zt[k][cb][:, nn * 512 : (nn + 1) * 512],
                                start=(k == 0),
                                stop=(k == 1),
                            )
                    ex = dpool.tile([128, CB], f32, tag="ex")
                    nc.scalar.activation(
                        out=ex[:],
                        in_=ps[:],
                        func=mybir.ActivationFunctionType.Exp,
                        bias=0.0,
                        scale=1.0 / TEMPERATURE,
                        accum_out=acc[:, m * N_CB + cb : m * N_CB + cb + 1],
                    )
                    # extract exp'd diagonals from the SBUF exp tile:
                    # self-sim diag lives in cb 0 at cols moff:moff+128,
                    # positive-pair diag in cb POS_CB at the same offset.
                    for cond, dst in ((cb == 0, sexp_t), (cb == POS_CB, pexp_t)):
                        if cond:
                            poff = moff  # 4096 % CB == 0: same offset in cb 0 and cb 2
                            sc = scpool.tile([128, 128], f32, tag="sc")
                            nc.vector.tensor_tensor(
                                sc[:],
                                ex[:, poff : poff + 128],
                                eye_t[:],
                                mybir.AluOpType.mult,
                            )
                            nc.vector.reduce_sum(
                                dst[:, m : m + 1], sc[:], axis=mybir.AxisListType.X
                            )
                    if cb == N_CB - 1:
                        # final rowsum for this row-tile as soon as its last
                        # column block is done — overlaps the kernel tail
                        nc.vector.reduce_sum(
                            s_t[:, m : m + 1],
                            acc[:, m * N_CB : (m + 1) * N_CB],
                            axis=mybir.AxisListType.X,
                        )

            nc.sync.dma_start(s_out[:], s_t[:])
            nc.sync.dma_start(sexp_out[:], sexp_t[:])
            nc.sync.dma_start(pexp_out[:], pexp_t[:])

    nc.compile()
    return nc


def _prepare_inputs(z1, z2):
    z = np.concatenate([np.asarray(z1), np.asarray(z2)], axis=0).astype(np.float32)
    norms = np.maximum(np.sqrt((z.astype(np.float64) ** 2).sum(-1)), EPS)
    zn = (z / norms[:, None]).astype(np.float32)
    znb = zn.astype(ml_dtypes.bfloat16)
    znt = np.ascontiguousarray(znb.T)  # [D, 2N]
    eye = np.eye(128, dtype=np.float32)
    in_maps = []
    for c in range(N_CORES):
        znt_c = np.ascontiguousarray(np.roll(znt, -c * ROWS_PER_CORE, axis=1))
        in_maps.append({"znt": znt_c, "eye": eye})
    return in_maps


def kernel(z1, z2):
    if "nc" not in _cached:
        _cached["nc"] = _build_bass()
    nc = _cached["nc"]
    in_maps = _prepare_inputs(z1, z2)
    res = run_bass_kernel_spmd(nc, in_maps, core_ids=list(range(N_CORES)))
    results = res.results

    per_row_loss = np.zeros(TWO_N, dtype=np.float64)
    for c in range(N_CORES):
        # [128, M_TILES]; element [l, m] belongs to global row c*1024 + m*128 + l
        S = np.asarray(results[c]["S"], dtype=np.float64)
        sexp = np.asarray(results[c]["sexp"], dtype=np.float64)
        pexp = np.asarray(results[c]["pexp"], dtype=np.float64)
        # drop the self-similarity term from the softmax denominator, then
        # loss_i = log(sum_{j!=i} exp(sim/T)) - pos/T
        rows = np.log((S - sexp).T.reshape(-1)) - np.log(pexp.T.reshape(-1))
        per_row_loss[c * ROWS_PER_CORE : (c + 1) * ROWS_PER_CORE] = rows
    return np.float32(per_row_loss.mean())
